# revision 5
# baseline (speedup 1.0000x reference)
"""Trainium2 Bass kernel for nn_Attention_42657615184259.

Multi-head attention block: x:[8,2048,384] -> qkv proj -> 6-head SDPA
(full softmax) -> out proj -> y:[8,2048,384].

Sharding: data-parallel over batch B=8, one batch element per NeuronCore.

Per-core design (everything in "transposed" space, contraction dims on
SBUF partitions):
  1. xT[c,n] built from x via PE transposes.
  2. qkT[j,n] = (qkv_w[:768] @ x.T) via f32r matmuls, stored bf16.
     v kept in natural layout v'[n, h, 0:64] (+ ones column at [:,h,64])
     so the AV matmul's ones-row yields softmax denominators for free.
  3. Per head h:
     scoresT[k,q] = kT.T @ qT (bf16 matmuls, PSUM f32)
     expT = exp(SCALE*scoresT) on ScalarE (PSUM->SBUF, bf16 out)
     out'[0:64,q] += v'_h.T @ expT accumulated over k chunks; row 64 = Z[q].
  4. Z rows gathered -> PE transpose -> reciprocal -> rT[n,h] (per-partition).
  5. proj per head with f32r matmuls; per-partition scale by rT; accumulate;
     + proj bias; DMA out.
"""

import os
import numpy as np
from contextlib import ExitStack

DIM = 384
HEADS = 6
DK = 64
N_TOK = 2048
B = 8
N_CORES = 8

_module_cache = {}


def build_module(n_tok=N_TOK, dim=DIM, heads=HEADS, debug=False):
    """Build + compile the per-core Bass module. Returns the Bacc object."""
    import concourse.bass as bass
    import concourse.tile as tile
    from concourse import bacc, mybir
    from concourse.masks import make_identity

    f32 = mybir.dt.float32
    bf16 = mybir.dt.bfloat16
    f32r = mybir.dt.float32r
    AF = mybir.ActivationFunctionType
    ALU = mybir.AluOpType

    assert dim % 128 == 0 and n_tok % 2048 == 0 and dim == heads * DK
    CC = dim // 128          # contraction chunks over model dim
    JC = 2 * dim // 128      # q+k row chunks
    NCH = n_tok // 128       # token chunks of 128
    NQ4 = n_tok // 512       # token chunks of 512
    HALF = n_tok // 2
    SCALE = DK ** -0.5

    nc = bacc.Bacc("TRN2", target_bir_lowering=False, debug=debug)

    x_d = nc.dram_tensor("x_b", [n_tok, dim], f32, kind="ExternalInput").ap()
    qkw_d = nc.dram_tensor("qkw_t", [dim, 2 * dim], f32r, kind="ExternalInput").ap()
    vw_d = nc.dram_tensor("vw_t", [dim, dim], f32r, kind="ExternalInput").ap()
    pw_d = nc.dram_tensor("pw_t", [dim, dim], f32r, kind="ExternalInput").ap()
    qkb_d = nc.dram_tensor("qk_b", [2 * dim], f32, kind="ExternalInput").ap()
    vb_d = nc.dram_tensor("v_b", [dim], f32, kind="ExternalInput").ap()
    pb_d = nc.dram_tensor("p_b", [dim], f32, kind="ExternalInput").ap()
    y_d = nc.dram_tensor("y_b", [n_tok, dim], f32, kind="ExternalOutput").ap()

    def r(ap):
        return ap.bitcast(f32r)

    with tile.TileContext(nc) as tc, ExitStack() as es:
        consts = es.enter_context(tc.tile_pool(name="consts", bufs=1))
        persist = es.enter_context(tc.tile_pool(name="persist", bufs=1))

        # ---- constants / weights ----
        qkwT = []
        vwT = []
        for cc in range(CC):
            t = consts.tile([128, 2 * dim], f32r, tag=f"qkw{cc}", name=f"qkw{cc}")
            nc.sync.dma_start(t, qkw_d[cc * 128:(cc + 1) * 128, :])
            qkwT.append(t)
            t = consts.tile([128, dim], f32r, tag=f"vw{cc}", name=f"vw{cc}")
            nc.sync.dma_start(t, vw_d[cc * 128:(cc + 1) * 128, :])
            vwT.append(t)
        pwT = []
        for h in range(heads):
            t = consts.tile([64, dim], f32r, tag=f"pw{h}", name=f"pw{h}")
            nc.sync.dma_start(t, pw_d[h * 64:(h + 1) * 64, :])
            pwT.append(t)
        qkb = []
        for jc in range(JC):
            t = consts.tile([128, 1], f32, tag=f"qkb{jc}", name=f"qkb{jc}")
            nc.sync.dma_start(t, qkb_d[jc * 128:(jc + 1) * 128])
            qkb.append(t)
        # free-axis biases broadcast across partitions via step-0 DMA
        vb_bc = consts.tile([128, dim], f32, tag="vb", name="vb")
        nc.gpsimd.dma_start(
            out=vb_bc,
            in_=bass.AP(tensor=vb_d.tensor, offset=vb_d.offset,
                        ap=[[0, 128], *vb_d.ap]),
        )
        pb_bc = consts.tile([128, dim], f32, tag="pb", name="pb")
        nc.gpsimd.dma_start(
            out=pb_bc,
            in_=bass.AP(tensor=pb_d.tensor, offset=pb_d.offset,
                        ap=[[0, 128], *pb_d.ap]),
        )
        ident = consts.tile([128, 128], f32, tag="ident", name="ident")
        make_identity(nc, ident)

        # persistent activations
        qkT = [persist.tile([128, n_tok], bf16, tag=f"qkT{jc}", name=f"qkT{jc}") for jc in range(JC)]
        vp = [persist.tile([128, heads, 65], bf16, tag=f"vp{ni}", name=f"vp{ni}") for ni in range(NCH)]

        # ---- phase A+B: xT, then qkT / v' ----
        with ExitStack() as es_xt:
            xtp = es_xt.enter_context(tc.tile_pool(name="xt", bufs=1))
            xT = [xtp.tile([128, n_tok], f32r, tag=f"xT{cc}", name=f"xT{cc}") for cc in range(CC)]
            with tc.tile_pool(name="xin", bufs=3) as xinp, \
                 tc.tile_pool(name="xtr", bufs=4, space="PSUM") as xtrp:
                for ni in range(NCH):
                    xin = xinp.tile([128, dim], f32, tag="xin", name="xin")
                    nc.sync.dma_start(xin, x_d[ni * 128:(ni + 1) * 128, :])
                    for cc in range(CC):
                        pt = xtrp.tile([128, 128], f32, tag="pt", name="pt")
                        nc.tensor.transpose(pt, xin[:, cc * 128:(cc + 1) * 128], ident)
                        nc.vector.tensor_copy(xT[cc][:, ni * 128:(ni + 1) * 128], pt)

            with tc.tile_pool(name="qkps", bufs=4, space="PSUM") as qkps, \
                 tc.tile_pool(name="vps", bufs=3, space="PSUM") as vps:
                for jc in range(JC):
                    for q4 in range(NQ4):
                        ps = qkps.tile([128, 512], f32, tag="qkps", name="qkps")
                        for cc in range(CC):
                            nc.tensor.matmul(
                                ps,
                                lhsT=qkwT[cc][:, jc * 128:(jc + 1) * 128],
                                rhs=xT[cc][:, q4 * 512:(q4 + 1) * 512],
                                start=(cc == 0), stop=(cc == CC - 1),
                            )
                        nc.vector.tensor_scalar_add(
                            qkT[jc][:, q4 * 512:(q4 + 1) * 512], ps, qkb[jc])
                for ni in range(NCH):
                    ps = vps.tile([128, dim], f32, tag="vps", name="vps")
                    for cc in range(CC):
                        nc.tensor.matmul(
                            ps,
                            lhsT=xT[cc][:, ni * 128:(ni + 1) * 128],
                            rhs=vwT[cc],
                            start=(cc == 0), stop=(cc == CC - 1),
                        )
                    nc.vector.tensor_tensor(
                        vp[ni][:, :, 0:64],
                        ps.rearrange("p (h d) -> p h d", h=heads),
                        vb_bc.rearrange("p (h d) -> p h d", h=heads),
                        ALU.add,
                    )
                    nc.vector.memset(vp[ni][:, :, 64:65], 1.0)
        # xT freed here

        attnT = [persist.tile([64, n_tok], f32r, tag=f"attnT{h}", name=f"attnT{h}") for h in range(heads)]
        zbuf = persist.tile([heads, n_tok], f32, tag="zbuf", name="zbuf")
        rT = persist.tile([128, NCH, heads], f32, tag="rT", name="rT")

        def qk_slice(row0, col0, ncols):
            """[64, ncols] slice of the qkT row space at row row0 (64-aligned)."""
            ti, po = divmod(row0, 128)
            return qkT[ti][po:po + 64, col0:col0 + ncols]

        # ---- phase C: attention per head ----
        with tc.tile_pool(name="sps", bufs=2, space="PSUM") as sps, \
             tc.tile_pool(name="avps", bufs=3, space="PSUM") as avps, \
             tc.tile_pool(name="expp", bufs=1) as expp, \
             tc.tile_pool(name="zst", bufs=2) as zstp:
            for h in range(heads):
                zstage = zstp.tile([65, n_tok], f32, tag="zst", name="zst")
                for half in range(2):
                    q0 = half * HALF
                    ets = []
                    for kc in range(NCH):
                        sp = sps.tile([128, 1024], f32, tag="sp", name="sp")
                        for qs in range(2):
                            nc.tensor.matmul(
                                sp[:, qs * 512:(qs + 1) * 512],
                                lhsT=qk_slice(dim + h * 64, kc * 128, 128),
                                rhs=qk_slice(h * 64, q0 + qs * 512, 512),
                                start=True, stop=True,
                            )
                        et = expp.tile([128, 1024], bf16, tag=f"e{kc}", name=f"e{kc}")
                        nc.scalar.activation(et, sp, AF.Exp, scale=SCALE)
                        ets.append(et)
                    for qs in range(2):
                        qc = half * 2 + qs
                        av = avps.tile([65, 512], f32, tag="av", name="av")
                        for kc in range(NCH):
                            nc.tensor.matmul(
                                av,
                                lhsT=vp[kc][:, h, :],
                                rhs=ets[kc][:, qs * 512:(qs + 1) * 512],
                                start=(kc == 0), stop=(kc == NCH - 1),
                            )
                        nc.vector.tensor_copy(
                            attnT[h][:, qc * 512:(qc + 1) * 512], av[0:64, :])
                        nc.vector.tensor_copy(
                            zstage[64:65, qc * 512:(qc + 1) * 512], av[64:65, :])
                # gather this head's Z row (cross-partition -> DMA)
                nc.sync.dma_start(zbuf[h:h + 1, :], zstage[64:65, :])

        # ---- phase D: normalizers + projection ----
        with tc.tile_pool(name="ztr", bufs=2, space="PSUM") as ztrp, \
             tc.tile_pool(name="projp", bufs=4, space="PSUM") as projp, \
             tc.tile_pool(name="ypool", bufs=3) as ypool, \
             tc.tile_pool(name="tmpp", bufs=3) as tmpp:
            for ni in range(NCH):
                zt = ztrp.tile([128, heads], f32, tag="zt", name="zt")
                nc.tensor.transpose(
                    zt, zbuf[:, ni * 128:(ni + 1) * 128], ident[0:heads, 0:heads])
                nc.vector.reciprocal(rT[:, ni, :], zt)
            for ni in range(NCH):
                yacc = ypool.tile([128, dim], f32, tag="yacc", name="yacc")
                for h in range(heads):
                    yp = projp.tile([128, dim], f32, tag="yp", name="yp")
                    nc.tensor.matmul(
                        yp,
                        lhsT=attnT[h][:, ni * 128:(ni + 1) * 128],
                        rhs=pwT[h],
                        start=True, stop=True,
                    )
                    if h == 0:
                        nc.vector.tensor_scalar(
                            yacc, yp, rT[:, ni, h:h + 1], None, ALU.mult)
                    else:
                        t = tmpp.tile([128, dim], f32, tag="tmp", name="tmp")
                        nc.vector.tensor_scalar(
                            t, yp, rT[:, ni, h:h + 1], None, ALU.mult)
                        nc.vector.tensor_add(yacc, yacc, t)
                yout = ypool.tile([128, dim], f32, tag="yout", name="yout")
                nc.vector.tensor_add(yout, yacc, pb_bc)
                nc.sync.dma_start(y_d[ni * 128:(ni + 1) * 128, :], yout)

    nc.compile()
    return nc


def make_in_maps(x, qkv_w, qkv_b, proj_w, proj_b, n_cores=N_CORES):
    """Host-side shard prep: per-core input dicts (weights host-transposed)."""
    x = np.asarray(x, dtype=np.float32)
    qkv_w = np.asarray(qkv_w, dtype=np.float32)
    qkv_b = np.asarray(qkv_b, dtype=np.float32)
    proj_w = np.asarray(proj_w, dtype=np.float32)
    proj_b = np.asarray(proj_b, dtype=np.float32)
    dim = x.shape[-1]
    shared = {
        "qkw_t": np.ascontiguousarray(qkv_w[:2 * dim].T),
        "vw_t": np.ascontiguousarray(qkv_w[2 * dim:3 * dim].T),
        "pw_t": np.ascontiguousarray(proj_w.T),
        "qk_b": np.ascontiguousarray(qkv_b[:2 * dim]),
        "v_b": np.ascontiguousarray(qkv_b[2 * dim:3 * dim]),
        "p_b": np.ascontiguousarray(proj_b),
    }
    return [
        {"x_b": np.ascontiguousarray(x[i]), **shared} for i in range(x.shape[0])
    ]


def run_on_hw(nc, in_maps, trace=False, trace_cores=None):
    from concourse import bass_utils
    return bass_utils.run_bass_kernel_spmd(
        nc, in_maps, core_ids=list(range(len(in_maps))),
        trace=trace, trace_cores=trace_cores,
    )


def kernel(x, qkv_w, qkv_b, proj_w, proj_b):
    key = (N_TOK, DIM, HEADS)
    if key not in _module_cache:
        _module_cache[key] = build_module(*key)
    nc = _module_cache[key]
    in_maps = make_in_maps(x, qkv_w, qkv_b, proj_w, proj_b)
    res = run_on_hw(nc, in_maps)
    y = np.stack([res.results[i]["y_b"] for i in range(len(in_maps))])
    return y.astype(np.float32)


if __name__ == "__main__":
    import reference
    inputs = reference.setup_inputs()
    out = kernel(**{k: np.asarray(v) for k, v in inputs.items()})
    print("out", out.shape, out.dtype)
